# revision 4
# baseline (speedup 1.0000x reference)
"""Causal multi-head attention (B=4, T=2048, H=16, hs=64, D=1024) on 8
Trainium2 NeuronCores — bf16 data-path variant.

Sharding: tensor-parallel over heads — each core computes 2 heads'
Q/K/V projections + attention, then a partial output projection
(y_partial = O_2h @ Wo[:, core_cols].T).  Host sums the 8 partials and
adds the bias.

All matmul operands are bf16 (PSUM accumulation stays fp32); element-wise
tiles are bf16 so the DVE runs in its 2x/4x packed modes, and the causal
masks are bf16 multiplies on the DVE (replacing the 2.1us GPSIMD
multiplies that sat on the exp->AV critical path).  x and y move over
HBM as bf16 (halved DMA).
"""

from contextlib import ExitStack

import numpy as np

import concourse.mybir as mybir
import concourse.tile as tile
from concourse import bacc

F32 = mybir.dt.float32
BF16 = mybir.dt.bfloat16
EXP = mybir.ActivationFunctionType.Exp

# problem shape (hardcoded per harness contract)
B, T, D, H, HS = 4, 2048, 1024, 16, 64
N_CORES = 8
HPC = H // N_CORES          # heads per core = 2
QB = 512                    # query block (matmul moving dim)
KC = 128                    # key chunk (partition dim)
SCALE = HS ** -0.5


def build_nc(b=B, t=T, d=D, hpc=HPC, loop_n=1, ablate=()):
    """Build the per-core program. All cores run the same NEFF; per-core
    data (weight slices) comes in through the input tensors."""
    n_dc = d // 128           # D chunks (contraction for projections)
    n_qb = t // QB            # query blocks
    n_kc = t // KC            # key chunks
    mh = 64 * hpc             # packed head width (=128 for hpc=2)

    nc = bacc.Bacc("TRN2", target_bir_lowering=False, debug=False)

    xT = nc.dram_tensor("xT", [b, d, t], BF16, kind="ExternalInput").ap()
    wq = nc.dram_tensor("wq", [d, mh], BF16, kind="ExternalInput").ap()
    wk = nc.dram_tensor("wk", [d, mh], BF16, kind="ExternalInput").ap()
    wv = nc.dram_tensor("wv", [d, mh], BF16, kind="ExternalInput").ap()
    woT = nc.dram_tensor("woT", [mh, d], BF16, kind="ExternalInput").ap()
    masks = nc.dram_tensor("masks", [2, KC, 2 * QB], BF16, kind="ExternalInput").ap()
    ident = nc.dram_tensor("ident", [128, 64], F32, kind="ExternalInput").ap()
    y = nc.dram_tensor("y", [b, t, d], BF16, kind="ExternalOutput").ap()

    with tile.TileContext(nc) as tc, ExitStack() as ctx:
        consts = ctx.enter_context(tc.tile_pool(name="consts", bufs=1))
        xt_pool = ctx.enter_context(tc.tile_pool(name="xt", bufs=n_dc))
        qkv_pool = ctx.enter_context(tc.tile_pool(name="qkv", bufs=2))
        vtil_pool = ctx.enter_context(tc.tile_pool(name="vtil", bufs=2 * hpc))
        p_pool = ctx.enter_context(tc.tile_pool(name="p", bufs=3))
        ot_pool = ctx.enter_context(tc.tile_pool(name="ot", bufs=2))
        ysb_pool = ctx.enter_context(tc.tile_pool(name="ysb", bufs=2))
        small_pool = ctx.enter_context(tc.tile_pool(name="small", bufs=2))

        ps_proj = ctx.enter_context(tc.tile_pool(name="psp", bufs=2, space="PSUM"))
        ps_s = ctx.enter_context(tc.tile_pool(name="pss", bufs=2, space="PSUM"))
        ps_av = ctx.enter_context(tc.tile_pool(name="psav", bufs=2, space="PSUM"))

        # --- constants ---
        wq_sb = consts.tile([128, n_dc, mh], BF16, tag="wq")
        wk_sb = consts.tile([128, n_dc, mh], BF16, tag="wk")
        wv_sb = consts.tile([128, n_dc, mh], BF16, tag="wv")
        for w_sb, w_dram in ((wq_sb, wq), (wk_sb, wk), (wv_sb, wv)):
            nc.sync.dma_start(w_sb[:], w_dram.rearrange("(c p) m -> p c m", p=128))
        woT_sb = consts.tile([mh, d], BF16, tag="wo")
        nc.sync.dma_start(woT_sb[:], woT[:])
        masks_sb = consts.tile([KC, 2, 2 * QB], BF16, tag="masks")
        nc.sync.dma_start(masks_sb[:], masks.rearrange("d p f -> p d f"))
        ident_sb = consts.tile([128, 64], F32, tag="ident")
        nc.sync.dma_start(ident_sb[:], ident[:])
        # ones column [128,1] for the Vtilde ones-column writes
        ones_bf = consts.tile([128, 1], BF16, tag="ones_bf")
        nc.vector.memset(ones_bf[:], 1.0)

        def make_proj_units(bi, st):
            """Load xT + QKV projections + Vtilde for batch bi, as a list
            of emission units (closures) to interleave with the previous
            batch's attention."""
            units = []

            def u_alloc():
                st["xt"] = []
                for c in range(n_dc):
                    xc = xt_pool.tile([128, t], BF16, tag="xt")
                    nc.sync.dma_start(xc[:], xT[bi, c * 128:(c + 1) * 128, :])
                    st["xt"].append(xc)
                st["qt2"] = qkv_pool.tile([mh, t], BF16, tag="qt2", name="qt2")
                st["kt2"] = qkv_pool.tile([mh, t], BF16, tag="kt2", name="kt2")
                st["vt2"] = qkv_pool.tile([mh, t], F32, tag="vt2", name="vt2")
            units.append(u_alloc)

            for key, wname in (("qt2", "wq"), ("kt2", "wk"), ("vt2", "wv")):
                for nb in range(n_qb):
                    def u_proj(key=key, wname=wname, nb=nb):
                        w_sb = {"wq": wq_sb, "wk": wk_sb, "wv": wv_sb}[wname]
                        dst = st[key]
                        acc = ps_proj.tile([mh, QB], F32, tag="proj")
                        for c in range(n_dc):
                            nc.tensor.matmul(
                                acc[:], w_sb[:, c, :],
                                st["xt"][c][:, nb * QB:(nb + 1) * QB],
                                start=(c == 0), stop=(c == n_dc - 1))
                        if key == "kt2":
                            nc.scalar.copy(dst[:, nb * QB:(nb + 1) * QB],
                                           acc[:])
                        else:
                            nc.vector.tensor_copy(
                                dst[:, nb * QB:(nb + 1) * QB], acc[:])
                    units.append(u_proj)

            for hh in range(hpc):
                def u_vtil(hh=hh):
                    vt = vtil_pool.tile([128, n_kc, 65], BF16, tag="vtil")
                    vt2 = st["vt2"]
                    # 8 transposes share one PSUM bank; a single strided
                    # copy evacuates all of them (fixed per-op DVE cost
                    # dominates, so fewer/bigger copies win)
                    for g in range(0, n_kc, 8):
                        gn = min(8, n_kc - g)
                        trp = ps_proj.tile([128, 512], F32, tag="proj")
                        for jj in range(gn):
                            j = g + jj
                            nc.tensor.transpose(
                                trp[:, jj * 64:(jj + 1) * 64],
                                vt2[hh * 64:(hh + 1) * 64,
                                    j * KC:(j + 1) * KC],
                                ident_sb[hh * 64:(hh + 1) * 64, :])
                        nc.vector.tensor_copy(
                            vt[:, g:g + gn, 0:64],
                            trp[:, 0:gn * 64].rearrange(
                                "p (j f) -> p j f", j=gn))
                    nc.vector.tensor_copy(
                        vt[:, :, 64], ones_bf[:].broadcast_to([128, n_kc]))
                    st[f"vtil{hh}"] = vt
                units.append(u_vtil)
            return units

        def make_attn_units(bi, st):
            """Attention + output projection for batch bi, one unit per
            query block."""
            units = []

            def u_attn(qb):
                if qb == 0:
                    st["ot"] = ot_pool.tile([mh, t], BF16, tag="ot", name="ot")
                ot_core = st["ot"]
                qt2, kt2 = st["qt2"], st["kt2"]
                kmax = (qb + 1) * (QB // KC)
                for hh in range(hpc):
                    qth = qt2[hh * 64:(hh + 1) * 64, :]
                    kth = kt2[hh * 64:(hh + 1) * 64, :]
                    vtil = st[f"vtil{hh}"]
                    oacc = ps_av.tile([128, QB], F32, tag="av")
                    for kc2 in range(kmax // 2):
                        # two score chunks share a 2-bank PSUM tile so one
                        # ACT instruction exps both (less per-op overhead)
                        sps = ps_s.tile([KC, 2 * QB], F32, tag="s")
                        for i in range(2):
                            kc = 2 * kc2 + i
                            nc.tensor.matmul(
                                sps[:, i * QB:(i + 1) * QB],
                                kth[:, kc * KC:(kc + 1) * KC],
                                qth[:, qb * QB:(qb + 1) * QB],
                                start=True, stop=True)
                        psb = p_pool.tile([KC, 2 * QB], BF16, tag="p")
                        nc.scalar.activation(psb[:], sps[:], EXP, scale=SCALE)
                        r = kc2 - 2 * qb
                        if r >= 0 and "no_mask" not in ablate:
                            # diagonal pair: one bf16 mask multiply on the
                            # DVE (4x packed mode) for both chunks
                            nc.vector.tensor_mul(psb[:], psb[:],
                                                 masks_sb[:, r, :])
                        for i in range(2):
                            kc = 2 * kc2 + i
                            nc.tensor.matmul(
                                oacc[0:65, :], vtil[:, kc, :],
                                psb[:, i * QB:(i + 1) * QB],
                                start=(kc == 0), stop=(kc == kmax - 1))
                    # normalise: recip of denom row, partition-broadcast
                    # (gpsimd), multiply into ot_core
                    recf = small_pool.tile([1, QB], F32, tag="recf")
                    nc.vector.reciprocal(recf[:], oacc[64:65, :])
                    bcs = small_pool.tile([64, QB], F32, tag="bcs")
                    nc.gpsimd.partition_broadcast(bcs[:], recf[:])
                    nc.vector.tensor_mul(
                        ot_core[hh * 64:(hh + 1) * 64, qb * QB:(qb + 1) * QB],
                        oacc[0:64, :], bcs[:])

                # output projection for this query block's T-chunks
                for tcn in range(qb * (QB // 128), (qb + 1) * (QB // 128)):
                    ysb = ysb_pool.tile([128, d], BF16, tag="ysb")
                    for nb0 in range(0, d, QB):
                        nw = min(QB, d - nb0)
                        op = ps_proj.tile([128, nw], F32, tag="proj")
                        nc.tensor.matmul(
                            op[:], ot_core[:, tcn * 128:(tcn + 1) * 128],
                            woT_sb[:, nb0:nb0 + nw],
                            start=True, stop=True)
                        if (tcn + nb0 // QB) % 2 == 0:
                            nc.vector.tensor_copy(ysb[:, nb0:nb0 + nw], op[:])
                        else:
                            nc.scalar.copy(ysb[:, nb0:nb0 + nw], op[:])
                    nc.sync.dma_start(y[bi, tcn * 128:(tcn + 1) * 128, :],
                                      ysb[:])

            for qb in range(n_qb):
                units.append(lambda qb=qb: u_attn(qb))
            return units

        def body():
            # software-pipelined emission: proj/load units of batch bi are
            # interleaved between the attention units of batch bi-1, so the
            # PE always has dense projection work to fill attention's
            # exp/mask dependency gaps.
            states = [dict() for _ in range(b)]
            prev_attn = None
            for bi in range(b):
                p_units = make_proj_units(bi, states[bi])
                a_units = make_attn_units(bi, states[bi])
                if prev_attn is None:
                    for u in p_units:
                        u()
                else:
                    # front-load next batch's proj into the early (cheap)
                    # attention blocks so proj(bi) is complete before the
                    # heavy last block of attn(bi-1); attn(bi) then overlaps
                    # that tail.
                    m = len(prev_attn)
                    k = len(p_units)
                    cuts = [0.4, 0.75, 1.0] + [1.0] * (m - 3)
                    emitted = 0
                    for j, au in enumerate(prev_attn):
                        au()
                        take = int(k * cuts[min(j, len(cuts) - 1)]) - emitted
                        for u in p_units[emitted:emitted + take]:
                            u()
                        emitted += take
                prev_attn = a_units
            for au in prev_attn:
                au()

        if loop_n > 1:
            with tc.For_i(0, loop_n, 1):
                body()
        else:
            body()

    nc.compile()
    return nc


_NC_CACHE = {}


def _get_nc():
    if "nc" not in _NC_CACHE:
        _NC_CACHE["nc"] = build_nc()
    return _NC_CACHE["nc"]


def make_masks() -> np.ndarray:
    """Two paired masks [KC, 2*QB]: pair 0 = [delta 0 | delta 128],
    pair 1 = [delta 256 | delta 384]."""
    m = np.zeros((2, KC, 2 * QB), np.float32)
    p = np.arange(KC)[:, None]
    f = np.arange(QB)[None, :]
    for pair in range(2):
        for half in range(2):
            dlt = (2 * pair + half) * KC
            m[pair][:, half * QB:(half + 1) * QB] = \
                (p + dlt <= f).astype(np.float32)
    return m


def make_in_maps(x, Wq, Wk, Wv, Wo):
    import ml_dtypes
    bf = ml_dtypes.bfloat16
    xTr = np.ascontiguousarray(x.transpose(0, 2, 1)).astype(bf)
    masks = make_masks().astype(bf)
    ident = np.tile(np.eye(64, dtype=np.float32), (2, 1))
    in_maps = []
    for c in range(N_CORES):
        h0 = c * HPC
        wq2 = Wq[h0:h0 + HPC].transpose(1, 0, 2).reshape(D, 64 * HPC).astype(bf)
        wk2 = Wk[h0:h0 + HPC].transpose(1, 0, 2).reshape(D, 64 * HPC).astype(bf)
        wv2 = Wv[h0:h0 + HPC].transpose(1, 0, 2).reshape(D, 64 * HPC).astype(bf)
        woT = np.ascontiguousarray(
            Wo[:, h0 * 64:(h0 + HPC) * 64].T).astype(bf)
        in_maps.append({
            "xT": xTr, "wq": wq2, "wk": wk2, "wv": wv2, "woT": woT,
            "masks": masks, "ident": ident,
        })
    return in_maps


def kernel(x, Wq, Wk, Wv, Wo, bo):
    from concourse.bass_utils import run_bass_kernel_spmd

    x = np.asarray(x, np.float32)
    in_maps = make_in_maps(x, np.asarray(Wq, np.float32),
                           np.asarray(Wk, np.float32),
                           np.asarray(Wv, np.float32),
                           np.asarray(Wo, np.float32))
    nc = _get_nc()
    res = run_bass_kernel_spmd(nc, in_maps, core_ids=list(range(N_CORES)))
    out = res.results[0]["y"].astype(np.float64)
    for c in range(1, N_CORES):
        out += res.results[c]["y"].astype(np.float64)
    out += np.asarray(bo, np.float64)
    return out.astype(np.float32)
